# revision 30
# baseline (speedup 1.0000x reference)
"""Trainium2 Bass kernel v6 for nn_BinLoss_7103875908252.

loss = mean_i ||features_i - centers[labels_i]||^2, labels from histogram
binning of target (edges = fp32 linspace(0,1,31) -> bin = ceil(30*v) for
v in [0,1); validated 0/32768 label mismatches vs searchsorted).

Data-parallel over 8 cores (4096 rows each), row-major layout
(row = p*32 + r). Per core:

  1. Binning on DVE ([128, 64] targets, trivial volume):
     b = ceil(30*v) = r + (v30 > r), r = f32(i32(v30)) -- robust to
     either cast rounding; lab = 32*b0 + b1 -> labi32 [128, 32] i32.
  2. 32 single-row indirect DMA gathers (one offset per partition per
     call is all the SWDGE ucode honors; ~1.5us/call Q7 pace) pull
     fp16 center rows from HBM into G16[p, r, :].
  3. Per chunk: DVE subtract diff = F32 - G16 (fp16 out), ACT
     Square+accum_out -- no tensor-engine work except the final
     partition reduce (ones-matmul), no PSUM pipeline.
  4. DVE reduce + ones-matmul -> [1,1] partial per core; host sums.

Engine budget per core: Q7 gather pace ~48us (the wall: per-row indexed
DMA descriptor generation is ~12ns/row on the one GpSimd engine), DMA
~12 MiB HBM, DVE ~24us, ACT ~30us, all overlapped behind the Q7 pace.
"""

import numpy as np

P = 128
D = 512
K = 1024
NCORES = 8
N = 32768
SHARD = N // NCORES          # 4096 rows per core
R = SHARD // P               # 32 rows per partition
NG = 4                       # gather calls (8 chunks each)
RPG = R // NG
NQ = 8                       # compute groups
RPQ = R // NQ                # 4 chunks per compute group

_CACHE = {}


def build_bass():
    import os
    from contextlib import ExitStack

    import concourse.bacc as bacc
    import concourse.tile as tile
    from concourse import bass, mybir

    f32 = mybir.dt.float32
    fp16 = mybir.dt.float16
    fp8 = mybir.dt.float8e4
    i32 = mybir.dt.int32
    A = mybir.AluOpType
    SQ = mybir.ActivationFunctionType.Square
    DRmode = mybir.MatmulPerfMode.DoubleRow
    NPE = 4                       # chunks 28..31 via PE one-hot

    nc = bacc.Bacc(
        "TRN2", target_bir_lowering=False, debug=False, num_devices=NCORES
    )
    feat = nc.dram_tensor("features", [SHARD, D], f32, kind="ExternalInput").ap()
    targ = nc.dram_tensor("target", [SHARD, 2], f32, kind="ExternalInput").ap()
    cent8 = nc.dram_tensor("cent8", [K, D], fp16, kind="ExternalInput").ap()
    onesd = nc.dram_tensor("ones1", [P, 1], f32, kind="ExternalInput").ap()
    centn8 = nc.dram_tensor("centn8", [K, D], fp8, kind="ExternalInput").ap()
    ioffd = nc.dram_tensor("iofull", [P, 8, 4, P], fp16, kind="ExternalInput").ap()
    idf32d = nc.dram_tensor("identf", [P, P], f32, kind="ExternalInput").ap()
    out = nc.dram_tensor("out", [1, 1], f32, kind="ExternalOutput").ap()

    DBG = bool(os.environ.get("KV5_DEBUG"))
    if DBG:
        d_lab = nc.dram_tensor("d_lab", [P, R], f32, kind="ExternalOutput").ap()
        d_g = nc.dram_tensor("d_g", [P, 2, D], fp16, kind="ExternalOutput").ap()
        d_acc = nc.dram_tensor("d_acc", [P, R], f32, kind="ExternalOutput").ap()

    with tile.TileContext(nc) as tc, ExitStack() as ctx:
        const_p = ctx.enter_context(tc.tile_pool(name="const", bufs=1))
        work_p = ctx.enter_context(tc.tile_pool(name="work", bufs=1))
        scr_p = ctx.enter_context(tc.tile_pool(name="scr", bufs=3))
        junk_p = ctx.enter_context(tc.tile_pool(name="junk", bufs=2))
        ps_p = ctx.enter_context(tc.tile_pool(name="ps", bufs=1, space="PSUM"))
        ps2_p = ctx.enter_context(tc.tile_pool(name="ps2", bufs=1, space="PSUM"))

        # ---- target tile FIRST on the sync ring: it gates binning ->
        # gathers; behind it, F packets would starve it for ~15us.
        T2 = work_p.tile([P, R, 2], f32)
        nc.sync.dma_start(T2[:], targ.rearrange("(p r) c -> p r c", p=P))

        # ---- feature stream (HWDGE sync ring): row-major, 8 KiB
        # contiguous per partition per call, 8 x 1 MiB for pipelining
        F32 = work_p.tile([P, R, D], f32)
        feat_re = feat.rearrange("(p r) d -> p r d", p=P)
        for g in range(8):
            nc.sync.dma_start(F32[:, 4 * g:4 * g + 4, :], feat_re[:, 4 * g:4 * g + 4, :])

        # ---- small consts on the scalar/ACT HWDGE ring -----------------
        ones1 = const_p.tile([P, 1], f32)
        nc.scalar.dma_start(ones1[:], onesd[:, :])
        idf32 = const_p.tile([P, P], f32)
        nc.scalar.dma_start(idf32[:], idf32d[:, :])
        ioff = const_p.tile([P, 8, 4, P], fp16)
        nc.scalar.dma_start(ioff[:], ioffd[:, :, :, :])
        C8n = const_p.tile([P, 8, D], fp8)
        nc.scalar.dma_start(C8n[:], centn8.rearrange("(c j) d -> j c d", j=P))

        # ACT Square table prefetch (overlaps DMA waits)
        dummy = const_p.tile([P, 1], fp16)
        nc.scalar.activation(out=dummy[:], in_=ones1[:], func=SQ)

        # ---- binning on DVE: labi32[p, r] = label(row p*32 + r) --------
        # ceil(x) = r + (x > r), r = float(int(x)); correct for either
        # trunc or round-to-nearest cast semantics.
        tv = T2[:].rearrange("p r c -> p (r c)")             # [128, 64]
        x = work_p.tile([P, 2 * R], f32)
        xi = work_p.tile([P, 2 * R], i32)
        xf = work_p.tile([P, 2 * R], f32)
        gt = work_p.tile([P, 2 * R], f32)
        b = work_p.tile([P, R, 2], f32)
        nc.vector.tensor_scalar(out=x[:], in0=tv, scalar1=30.0, scalar2=None, op0=A.mult)
        nc.vector.tensor_copy(out=xi[:], in_=x[:])
        nc.vector.tensor_copy(out=xf[:], in_=xi[:])
        nc.vector.tensor_tensor(out=gt[:], in0=x[:], in1=xf[:], op=A.is_gt)
        nc.vector.tensor_tensor(
            out=b[:].rearrange("p r c -> p (r c)"), in0=xf[:], in1=gt[:], op=A.add
        )
        labm = work_p.tile([P, R], f32)
        lab = work_p.tile([P, R], f32)
        nc.vector.tensor_scalar(
            out=labm[:], in0=b[:, :, 0], scalar1=32.0, scalar2=None, op0=A.mult
        )
        nc.vector.tensor_tensor(out=lab[:], in0=labm[:], in1=b[:, :, 1], op=A.add)
        labi = work_p.tile([P, R], i32)
        nc.vector.tensor_copy(out=labi[:], in_=lab[:])
        if DBG:
            nc.sync.dma_start(d_lab[:, :], lab[:])

        # ---- gathers: 32 single-row indirect DMAs (the HW ucode only
        # honors ONE offset per partition per call), interleaved with
        # per-chunk subtract + square so compute tracks the Q7 pace.
        G8 = work_p.tile([P, R, D], fp16)
        acc = work_p.tile([P, R], f32)
        for r in range(R - NPE):
            nc.gpsimd.indirect_dma_start(
                out=G8[:, r, :],
                out_offset=None,
                in_=cent8[:, :],
                in_offset=bass.IndirectOffsetOnAxis(ap=labi[:, r:r + 1], axis=0),
            )

        # ---- PE one-hot path for the last NPE chunks (no Q7 cost):
        # Sel[k, i] one-hot of lab via broadcast-transpose + is_equal,
        # PSUM = sum_k Sel*(-C8) (fp8 DoubleRow), dif = F + PSUM on DVE.
        psB = ps_p.tile([P, NPE, P], f32, tag="psB")
        for t in range(NPE):
            nc.tensor.transpose(
                out=psB[:, t, :],
                in_=lab[:, R - NPE + t].to_broadcast([P, P]),
                identity=idf32[:],
            )
        labT4 = work_p.tile([P, NPE, P], fp16)
        nc.vector.tensor_copy(out=labT4[:], in_=psB[:])
        pe_ps = {}
        for t in range(NPE):
            sel = work_p.tile([P, 8, P], fp8, tag=f"sel{t}")
            nc.vector.tensor_tensor(
                out=sel[:],
                in0=labT4[:, t, :].unsqueeze(1).broadcast_to([P, 8, P]),
                in1=ioff[:, :, t, :],
                op=A.is_equal,
            )
            pst = ps2_p.tile([P, D], f32, tag=f"pe{t}")
            pe_ps[t] = pst
            for q in range(4):
                nc.tensor.matmul(
                    out=pst[:],
                    lhsT=sel[:, 2 * q:2 * q + 2, :],
                    rhs=C8n[:, 2 * q:2 * q + 2, :],
                    start=(q == 0), stop=(q == 3),
                    perf_mode=DRmode,
                )
        if DBG:
            gdbg = work_p.tile([P, 2, D], fp16)
            nc.vector.tensor_copy(out=gdbg[:], in_=G8[:, 0:2, :])
            nc.sync.dma_start(d_g[:, :, :], gdbg[:])

        for r in range(R):
            dif = scr_p.tile([P, D], fp16, tag="dif")
            if r < R - NPE:
                nc.vector.tensor_tensor(
                    out=dif[:], in0=F32[:, r, :], in1=G8[:, r, :], op=A.subtract
                )
            else:
                nc.vector.tensor_tensor(
                    out=dif[:], in0=F32[:, r, :], in1=pe_ps[r - (R - NPE)][:],
                    op=A.add,
                )
            sq = junk_p.tile([P, D], fp16, tag="sq")
            nc.scalar.activation(
                out=sq[:], in_=dif[:], func=SQ, accum_out=acc[:, r:r + 1]
            )
        if DBG:
            nc.sync.dma_start(d_acc[:, :], acc[:])

        # ---- final reduction -------------------------------------------
        s = work_p.tile([P, 1], f32)
        nc.vector.tensor_reduce(
            out=s[:], in_=acc[:], axis=mybir.AxisListType.X, op=A.add
        )
        psf = ps_p.tile([1, 1], f32, tag="fin")
        nc.tensor.matmul(out=psf[:], lhsT=ones1[:], rhs=s[:], start=True, stop=True)
        res = work_p.tile([1, 1], f32)
        nc.vector.tensor_copy(out=res[:], in_=psf[:])
        nc.sync.dma_start(out[:, :], res[:])

    nc.compile()
    return nc


def _consts():
    iofull = np.broadcast_to(
        (np.arange(P)[:, None, None, None]
         + 128.0 * np.arange(8)[None, :, None, None]),
        (P, 8, 4, P),
    ).astype(np.float16)
    return dict(
        ones1=np.ones((P, 1), dtype=np.float32),
        iofull=np.ascontiguousarray(iofull),
        identf=np.eye(P, dtype=np.float32),
    )


def _cent8(centers):
    return np.ascontiguousarray(np.asarray(centers, dtype=np.float32).astype(
        np.float16
    ))


def _get_nc():
    if "nc" not in _CACHE:
        _CACHE["nc"] = build_bass()
    return _CACHE["nc"]


def kernel(features, target, centers):
    from concourse.bass_utils import run_bass_kernel_spmd

    features = np.ascontiguousarray(features, dtype=np.float32)
    target = np.ascontiguousarray(target, dtype=np.float32)
    import ml_dtypes
    cent8 = _cent8(centers)
    centn8 = np.ascontiguousarray(
        (-np.asarray(centers, dtype=np.float32)).astype(ml_dtypes.float8_e4m3)
    )
    consts = _consts()

    nc = _get_nc()
    in_maps = []
    for c in range(NCORES):
        sl = slice(c * SHARD, (c + 1) * SHARD)
        in_maps.append(
            {
                "features": np.ascontiguousarray(features[sl]),
                "target": np.ascontiguousarray(target[sl]),
                "cent8": cent8,
                "centn8": centn8,
                **consts,
            }
        )
    r = run_bass_kernel_spmd(
        nc,
        in_maps,
        core_ids=list(range(NCORES)),
        trace=_CACHE.get("trace", False),
        tmpdir=_CACHE.get("tmpdir"),
    )
    _CACHE["last_results"] = r
    total = sum(float(res["out"][0, 0]) for res in r.results)
    return np.float32(total / N)
